# revision 8
# baseline (speedup 1.0000x reference)
"""CRF log-partition (forward algorithm) kernel for 8 TRN2 NeuronCores.

Math
----
reference:  s = score[:, 1:-1, :]  [B, T, L], T=2048, L=48
  alpha_t[i] = s_t[i] + logsumexp_j(trans[i,j] + alpha_{t-1}[j]),  alpha_0 = onehot(BOS)
  out[b] = logsumexp_i(alpha_T[i] + trans[EOS, i])

Exp domain:  E = exp(trans), x_t = exp(s_t), p_t = diag(x_t) E p_{t-1}
  Z = f^T D_T E D_{T-1} E ... D_1 E p_0,   f = exp(trans[EOS, :])

Meet in the middle (M = T/2):
  fwd:  p_k = x_k o (E p_{k-1}),                    k = 1..M
  bwd:  r_t = x_t o (E^T r_{t+1}),  r_T = x_T o f,  t = T-1 .. M+1
  r' = E^T r_{M+1}       (one extra E^T application, using x == 1)
  Z  = sum_i p_M[i] * r'[i]

Device layout (per core, batch shard BC=32), partitions:
  rows 0:2    renorm scratch: column sums land here (psum), reciprocals +
              rank-1 broadcast read from here.  State/x rows kept == 0 here.
  rows 2:50   fwd state p / fwd x
  rows 50:98  bwd state r / bwd x

One matmul per merged step with block-diagonal stationary W [98, 98]
(E^T block fwd, E block bwd, ones-columns computing column sums into psum
rows 0:2, ones-rows for the renorm broadcast).  One DVE tensor_tensor
multiply applies x (pre-exponentiated in bulk by ScalarE from DMA-streamed
raw scores).  Every RENORM steps: reciprocal of colsums + broadcast matmul +
one extra multiply; reciprocals go to DRAM, log corrections applied on host.
E is pre-scaled by BETA to center per-step growth at ~1.
"""

import sys

sys.path.insert(0, "/opt/trn_rl_repo")

import numpy as np

import concourse.bacc as bacc
import concourse.tile as tile
from concourse import mybir

L = 48
NCORES = 8
BOS_IDX = 0
EOS_IDX = 1
BETA = float(1.0 / (48.0 * np.e))
NEG = -10000.0
F0 = 2        # fwd block row offset
B0 = 2 + L    # bwd block row offset
P = 2 + 2 * L # 98 partitions

LAST_EXEC_NS = None

_NC_CACHE = {}


def build_nc(TH, BC, renorm=32, wch=64, debug=False):
    """Build + compile the per-core Bass graph."""
    assert TH % renorm == 0 and TH % wch == 0
    nren = TH // renorm
    nch = TH // wch
    f32 = mybir.dt.float32
    Exp = mybir.ActivationFunctionType.Exp
    mult = mybir.AluOpType.mult

    nc = bacc.Bacc("TRN2", target_bir_lowering=False, debug=debug)

    xs_d = nc.declare_dram_parameter("xs", [P, TH * BC], f32, isOutput=False)
    vinit_d = nc.declare_dram_parameter("vinit", [P, BC], f32, isOutput=False)
    w_d = nc.declare_dram_parameter("wmat", [P, P], f32, isOutput=False)
    outv_d = nc.declare_dram_parameter("outv", [P, BC], f32, isOutput=True)
    recs_d = nc.declare_dram_parameter("recs", [2, nren * BC], f32, isOutput=True)

    with tile.TileContext(nc) as tc:
        with (
            tc.tile_pool(name="singles", bufs=1) as singles,
            tc.tile_pool(name="xraw", bufs=3) as xraw_pool,
            tc.tile_pool(name="xexp", bufs=nch) as xexp_pool,
            tc.tile_pool(name="vbuf", bufs=3) as vpool,
            tc.tile_pool(name="psA", bufs=4, space="PSUM") as psA,
            tc.tile_pool(name="psB", bufs=2, space="PSUM") as psB,
        ):
            w_sb = singles.tile([P, P], f32)
            nc.sync.dma_start(w_sb[:], w_d[:])

            # renorm reciprocals, packed along the free dim at partitions 0:2
            recs_sb = singles.tile([2, nren * BC], f32)

            vts = [
                vpool.tile([P, BC], f32, tag="v", name=f"v{i}") for i in range(3)
            ]
            nc.sync.dma_start(vts[0][:], vinit_d[:])

            # stream raw scores in, exponentiate in bulk on ScalarE
            xch = []
            for c in range(nch):
                xr = xraw_pool.tile([P, wch * BC], f32, tag="xr", name=f"xr{c}")
                nc.sync.dma_start(
                    xr[:], xs_d[:, c * wch * BC : (c + 1) * wch * BC]
                )
                xe = xexp_pool.tile([P, wch * BC], f32, tag="xe", name=f"xe{c}")
                nc.scalar.activation(xe[:], xr[:], Exp)
                xch.append(xe)

            vi = 0
            ren = 0
            for k in range(TH):
                xk = xch[k // wch][:, (k % wch) * BC : (k % wch + 1) * BC]
                pt = psA.tile([P, BC], f32, tag="ps", name=f"ps{k}")
                nc.tensor.matmul(
                    pt[:], w_sb[:, :], vts[vi][:], start=True, stop=True
                )
                nxt = vts[(vi + 1) % 3]
                nc.vector.tensor_tensor(nxt[:], pt[:], xk, mult)
                if (k + 1) % renorm == 0:
                    rslice = recs_sb[:, ren * BC : (ren + 1) * BC]
                    nc.vector.reciprocal(rslice, pt[0:2, :])
                    pb = psB.tile([P, BC], f32, tag="pb", name=f"pb{ren}")
                    nc.tensor.matmul(
                        pb[:], w_sb[0:2, :], rslice, start=True, stop=True
                    )
                    nxt2 = vts[(vi + 2) % 3]
                    nc.vector.tensor_tensor(nxt2[:], nxt[:], pb[:], mult)
                    vi = (vi + 2) % 3
                    ren += 1
                else:
                    vi = (vi + 1) % 3

            nc.sync.dma_start(outv_d[:], vts[vi][:])
            nc.sync.dma_start(recs_d[:], recs_sb[:])

    nc.compile()
    return nc


def get_nc(TH, BC, renorm=32, wch=64):
    key = (TH, BC, renorm, wch)
    if key not in _NC_CACHE:
        _NC_CACHE[key] = build_nc(TH, BC, renorm=renorm, wch=wch)
    return _NC_CACHE[key]


def make_wmat(trans):
    """Stationary matrix [P, P] (lhsT layout: [K rows, M cols])."""
    Ebar = (BETA * np.exp(trans.astype(np.float64))).astype(np.float32)
    W = np.zeros((P, P), np.float32)
    # fwd block: out[i] = sum_j E[i,j] v[j]  ->  lhsT[F0+j, F0+i] = E[i, j]
    W[F0 : F0 + L, F0 : F0 + L] = Ebar.T
    # bwd block: out[i] = sum_j E[j,i] v[j]  ->  lhsT[B0+j, B0+i] = E[j, i]
    W[B0 : B0 + L, B0 : B0 + L] = Ebar
    # column sums of fwd / bwd state -> psum rows 0 / 1
    W[F0 : F0 + L, 0] = 1.0
    W[B0 : B0 + L, 1] = 1.0
    # rank-1 broadcast rows for renorm (moving operand at partitions 0:2)
    W[0, F0 : F0 + L] = 1.0
    W[1, B0 : B0 + L] = 1.0
    return W


def make_core_inputs(s_shard, trans, TH):
    """s_shard: [BC, T, L] stripped scores -> (xs [P, TH, BC], vinit [P, BC])."""
    BC, T, Lx = s_shard.shape
    assert T == 2 * TH and Lx == L
    xs = np.full((P, TH, BC), NEG, np.float32)  # exp(NEG) == 0 filler
    # fwd merged step k0 (0-based) applies x_{k0+1} = exp(s[:, k0, :])
    xs[F0 : F0 + L] = np.ascontiguousarray(s_shard[:, 0:TH, :].transpose(2, 1, 0))
    # bwd merged step k0 applies x_{T-1-k0} = exp(s[:, T-2-k0, :]) for k0 < TH-1
    if TH > 1:
        xs[B0 : B0 + L, 0 : TH - 1] = np.ascontiguousarray(
            s_shard[:, T - 2 : TH - 1 : -1, :].transpose(2, 1, 0)
        )
    # last bwd step multiplies by exp(0) = 1
    xs[B0 : B0 + L, TH - 1] = 0.0
    vinit = np.zeros((P, BC), np.float32)
    vinit[F0 + BOS_IDX, :] = 1.0
    rT = np.exp(
        trans.astype(np.float64)[EOS_IDX][None, :]
        + s_shard[:, T - 1, :].astype(np.float64)
    )
    vinit[B0 : B0 + L] = rT.T.astype(np.float32)
    return xs, vinit


def finish_host(outv, recs, TH, BC, nren):
    v = outv.astype(np.float64)
    rc = recs.astype(np.float64).reshape(2, nren, BC)
    z = (v[F0 : F0 + L] * v[B0 : B0 + L]).sum(axis=0)
    return np.log(z) - np.log(rc).sum(axis=(0, 1)) - 2.0 * TH * np.log(BETA)


# ---------------------------------------------------------------------------
# Cached PJRT runner (mirrors bass2jax.run_bass_via_pjrt multi-core path, but
# caches the compiled executable and supports device-resident inputs).
# ---------------------------------------------------------------------------

_RUN_CACHE = {}


def _get_runner(nc, n_cores):
    key = id(nc)
    if key in _RUN_CACHE:
        return _RUN_CACHE[key]

    import jax
    from jax.sharding import Mesh, PartitionSpec
    from jax.experimental.shard_map import shard_map
    from concourse.bass2jax import (
        _bass_exec_p,
        install_neuronx_cc_hook,
        partition_id_tensor,
    )

    install_neuronx_cc_hook()
    partition_name = (
        nc.partition_id_tensor.name if nc.partition_id_tensor is not None else None
    )
    in_names, out_names, out_avals, zero_outs = [], [], [], []
    for alloc in nc.m.functions[0].allocations:
        if not isinstance(alloc, mybir.MemoryLocationSet):
            continue
        name = alloc.memorylocations[0].name
        if alloc.kind == "ExternalInput":
            if name != partition_name:
                in_names.append(name)
        elif alloc.kind == "ExternalOutput":
            out_names.append(name)
            shape = tuple(alloc.tensor_shape)
            dtype = mybir.dt.np(alloc.dtype)
            out_avals.append(jax.core.ShapedArray(shape, dtype))
            zero_outs.append(np.zeros(shape, dtype))
    n_params = len(in_names)
    n_outs = len(out_avals)
    all_in_names = in_names + out_names
    if partition_name is not None:
        all_in_names = all_in_names + [partition_name]

    def _body(*args):
        operands = list(args)
        if partition_name is not None:
            operands.append(partition_id_tensor())
        return tuple(
            _bass_exec_p.bind(
                *operands,
                out_avals=tuple(out_avals),
                in_names=tuple(all_in_names),
                out_names=tuple(out_names),
                lowering_input_output_aliases=(),
                sim_require_finite=True,
                sim_require_nnan=True,
                nc=nc,
            )
        )

    devices = jax.devices()[:n_cores]
    mesh = Mesh(np.asarray(devices), ("core",))
    fn = jax.jit(
        shard_map(
            _body,
            mesh=mesh,
            in_specs=(PartitionSpec("core"),) * (n_params + n_outs),
            out_specs=(PartitionSpec("core"),) * n_outs,
            check_rep=False,
        )
    )
    runner = {
        "fn": fn,
        "in_names": in_names,
        "out_names": out_names,
        "out_avals": out_avals,
        "concat_zeros": [
            np.zeros((n_cores * z.shape[0], *z.shape[1:]), z.dtype)
            for z in zero_outs
        ],
        "n_cores": n_cores,
        "jax": jax,
    }
    _RUN_CACHE[key] = runner
    return runner


def _prep_dev_args(runner, in_maps):
    jax = runner["jax"]
    concat_in = [
        np.concatenate([np.asarray(m[name]) for m in in_maps], axis=0)
        for name in runner["in_names"]
    ]
    return [jax.device_put(a) for a in concat_in] + [
        jax.device_put(z) for z in runner["concat_zeros"]
    ]


def _execute(runner, dev_args):
    jax = runner["jax"]
    out = runner["fn"](*dev_args)
    jax.block_until_ready(out)
    return out


def _results_per_core(runner, out_arrs):
    n_cores = runner["n_cores"]
    return [
        {
            name: np.asarray(out_arrs[i]).reshape(
                n_cores, *runner["out_avals"][i].shape
            )[c]
            for i, name in enumerate(runner["out_names"])
        }
        for c in range(n_cores)
    ]


LAST_STATE = {}


def kernel(score, trans):
    global LAST_EXEC_NS
    score = np.asarray(score, dtype=np.float32)
    trans = np.asarray(trans, dtype=np.float32)
    B, TF, Lx = score.shape
    T = TF - 2
    TH = T // 2
    BC = B // NCORES
    renorm, wch = 32, 64
    nren = TH // renorm

    s = score[:, 1:-1, :]
    W = make_wmat(trans)
    in_maps = []
    for c in range(NCORES):
        xs, vinit = make_core_inputs(s[c * BC : (c + 1) * BC], trans, TH)
        in_maps.append({"xs": xs.reshape(P, -1), "vinit": vinit, "wmat": W})

    nc = get_nc(TH, BC, renorm=renorm, wch=wch)
    runner = _get_runner(nc, NCORES)
    dev_args = _prep_dev_args(runner, in_maps)
    out_arrs = _execute(runner, dev_args)
    results = _results_per_core(runner, out_arrs)
    LAST_STATE.update(runner=runner, dev_args=dev_args)

    outs = []
    for c in range(NCORES):
        logZ = finish_host(results[c]["outv"], results[c]["recs"], TH, BC, nren)
        outs.append(logZ.astype(np.float32))
    return np.concatenate(outs)


def time_exec(n=10):
    """Re-execute the last kernel invocation n times; return per-call wall
    times in ns (device-resident inputs, compiled executable)."""
    import time

    runner, dev_args = LAST_STATE["runner"], LAST_STATE["dev_args"]
    _execute(runner, dev_args)  # warm
    times = []
    for _ in range(n):
        t0 = time.perf_counter()
        _execute(runner, dev_args)
        times.append((time.perf_counter() - t0) * 1e9)
    return times
